# revision 8
# baseline (speedup 1.0000x reference)
"""AutoCorrelation (Autoformer-style) kernel for 8 TRN2 NeuronCores.

Reference math (B=4, L=2048, D=1024, H=16, DK=64):
  qp = q @ Wq + bq ; kp = k @ Wk + bk ; vp = v @ Wv + bv          # [B, L, D]
  corr = irfft(rfft(qp, t) * conj(rfft(kp, t)))                   # circular corr
  m[c] = max_{tau in 1..L-1} corr[tau, c]                         # top-1 value
  per head: w = softmax_{dk}(m); shift_c = int(w_c) in {0, 1}     # 1 iff w_c
                                                                  # rounds to 1.0f
  out = roll(vp, shift, axis=t) @ Wo + bo

Only the top-1 correlation value feeds the output (reference takes
delays[:, 0]), so top-k reduces to a max. Shifts are 0 except when a
softmax weight rounds to exactly 1.0f (sum of the other exp terms
< 2^-24), which happens for a handful of heads.

Sharding: core = (b, j): batch b = core//2, channel half j = core%2
(8 heads = 512 channels). Projections use column/row slices of the
weights; FFT/max/roll are head-local; the output projection produces
partial sums over the 512-channel slice which the host reduces.

DFTs are dense matmuls against precomputed trig matrices (rfft bins
1025 padded to 1152 = 9*128 partitions). All matmuls run in float32r
(~14-bit mantissa at full PE rate); the shift-decision margin for this
input is ~4 sigma above the resulting noise and is verified by test.py
against the jax reference decisions.

The host passes q/k/v pre-transposed ([D, L] per batch) so the
contraction dim of every matmul lands on SBUF partitions without
on-device transposes.
"""

import math

import numpy as np

import concourse.bacc as bacc
import concourse.mybir as mybir
from concourse import tile
from concourse.bass_utils import run_bass_kernel_spmd

B, L, D, H, DK = 4, 2048, 1024, 16, 64
NCORES = 8
CPC = D // 2        # channels per core (8 heads)
NBINS = L // 2 + 1  # 1025 rfft bins
FPAD = 1024         # Nyquist bin packed into the f=0 sin slot
FT = FPAD // 128    # 9
TT = L // 128       # 16
NT = L // 512       # 4
CT = CPC // 128     # 4
DT = D // 128       # 8
F32 = mybir.dt.float32
F32R = mybir.dt.float32r
AX = mybir.AxisListType.X
OP = mybir.AluOpType


def build():
    nc = bacc.Bacc("TRN2", target_bir_lowering=False, debug=False, num_devices=NCORES)

    dram = {}
    for name, shape, dt in [
        ("qT", [D, L], F32R), ("kT", [D, L], F32R), ("vT", [D, L], F32R),
        ("wq", [D, CPC], F32R), ("wk", [D, CPC], F32R), ("wv", [D, CPC], F32R),
        ("wo", [CPC, D], F32R),
        ("cf", [L, FPAD], F32R), ("cfs", [L, FPAD], F32R),
        ("cic", [FPAD, L], F32R), ("cis", [FPAD, L], F32R),
        ("bq8", [8, DK], F32), ("bk8", [8, DK], F32),
    ]:
        dram[name] = nc.dram_tensor(name, shape, dt, kind="ExternalInput").ap()
    out_d = nc.dram_tensor("out", [L, D], F32, kind="ExternalOutput").ap()
    m8_d = nc.dram_tensor("m8dbg", [8, DK], F32, kind="ExternalOutput").ap()
    fl_d = nc.dram_tensor("flag8dbg", [8, DK], F32, kind="ExternalOutput").ap()

    with tile.TileContext(nc) as tc:
        with tc.tile_pool(name="small", bufs=1) as smp:
            fq08 = smp.tile([8, DK], F32, tag="fq08")
            fk08 = smp.tile([8, DK], F32, tag="fk08")
            mx4 = [smp.tile([128, NT], F32, tag=f"mx4_{c}", name=f"mx4_{c}") for c in range(CT)]
            m_t = [smp.tile([128, 1], F32, tag=f"m{c}", name=f"m{c}") for c in range(CT)]

            with tc.tile_pool(name="pp", bufs=1) as ppool, \
                 tc.tile_pool(name="cip0", bufs=1) as cip0:
                pr_t = [ppool.tile([128, CPC], F32R, tag=f"pr{f}", name=f"pr{f}") for f in range(FT)]
                pi_t = [ppool.tile([128, CPC], F32R, tag=f"pi{f}", name=f"pi{f}") for f in range(FT)]

                with tc.tile_pool(name="qkp", bufs=1) as qkp, \
                     tc.tile_pool(name="cf2", bufs=2) as cfp:
                    cic0_t = [cip0.tile([128, 512], F32R, tag=f"cic0_{f}", name=f"cic0_{f}") for f in range(FT)]
                    for f in range(FT):
                        nc.scalar.dma_start(out=cic0_t[f][:],
                                            in_=dram["cic"][f * 128:(f + 1) * 128, 0:512])
                    qp_t = [qkp.tile([128, CPC], F32R, tag=f"qp{t}", name=f"qp{t}") for t in range(TT)]
                    kp_t = [qkp.tile([128, CPC], F32R, tag=f"kp{t}", name=f"kp{t}") for t in range(TT)]

                    # prefetch the first forward-trig columns during stage 1
                    cf_tiles = {}
                    def load_cf(f):
                        cfc = cfp.tile([128, L], F32R, tag="cfc", name=f"cfc{f}")
                        cfs_ = cfp.tile([128, L], F32R, tag="cfs", name=f"cfs{f}")
                        nc.sync.dma_start(
                            out=cfc[:],
                            in_=dram["cf"][:, f * 128:(f + 1) * 128].rearrange("(n p) f -> p n f", p=128))
                        nc.scalar.dma_start(
                            out=cfs_[:],
                            in_=dram["cfs"][:, f * 128:(f + 1) * 128].rearrange("(n p) f -> p n f", p=128))
                        cf_tiles[f] = (cfc, cfs_)
                    load_cf(0)

                    # ---- stage 1: projections qp/kp in [t, c] layout -----
                    with tc.tile_pool(name="w1", bufs=1) as w1, \
                         tc.tile_pool(name="xcol", bufs=2) as xcol, \
                         tc.tile_pool(name="ps1", bufs=2, space="PSUM") as ps1:
                        wq_t = [w1.tile([128, CPC], F32R, tag=f"wq{d}", name=f"wq{d}") for d in range(DT)]
                        wk_t = [w1.tile([128, CPC], F32R, tag=f"wk{d}", name=f"wk{d}") for d in range(DT)]
                        for d in range(DT):
                            nc.scalar.dma_start(out=wq_t[d][:], in_=dram["wq"][d * 128:(d + 1) * 128, :])
                            nc.scalar.dma_start(out=wk_t[d][:], in_=dram["wk"][d * 128:(d + 1) * 128, :])
                        for t in range(TT):
                            qc = xcol.tile([128, D], F32R, tag="qcol")
                            kc = xcol.tile([128, D], F32R, tag="kcol")
                            nc.sync.dma_start(
                                out=qc[:],
                                in_=dram["qT"][:, t * 128:(t + 1) * 128].rearrange("(n p) t -> p n t", p=128))
                            nc.scalar.dma_start(
                                out=kc[:],
                                in_=dram["kT"][:, t * 128:(t + 1) * 128].rearrange("(n p) t -> p n t", p=128))
                            pq = ps1.tile([128, CPC], F32, tag="pq")
                            pk = ps1.tile([128, CPC], F32, tag="pk")
                            for d in range(DT):
                                nc.tensor.matmul(pq[:], qc[:, d * 128:(d + 1) * 128], wq_t[d][:],
                                                 start=(d == 0), stop=(d == DT - 1))
                            for d in range(DT):
                                nc.tensor.matmul(pk[:], kc[:, d * 128:(d + 1) * 128], wk_t[d][:],
                                                 start=(d == 0), stop=(d == DT - 1))
                            nc.vector.tensor_copy(qp_t[t][:], pq[:])
                            nc.vector.tensor_copy(kp_t[t][:], pk[:])

                    # ---- stage 2+3: forward DFT and cross-spectrum P -----
                    # cfs column 0 carries (-1)^t (Nyquist row packed into the
                    # zero sin(0) slot); P row 0 is fixed up below.
                    with tc.tile_pool(name="fq2", bufs=2) as fqp, \
                         tc.tile_pool(name="ps2", bufs=2, space="PSUM") as ps2:
                        for f in range(FT):
                            if f + 1 < FT:
                                load_cf(f + 1)
                            cfc, cfs_ = cf_tiles.pop(f)
                            ps = {}
                            for nm, src, trig in (("qr", qp_t, cfc), ("qi", qp_t, cfs_),
                                                  ("kr", kp_t, cfc), ("ki", kp_t, cfs_)):
                                p = ps2.tile([128, CPC], F32, tag=nm, name="ps2" + nm)
                                for t in range(TT):
                                    nc.tensor.matmul(p[:], trig[:, t * 128:(t + 1) * 128], src[t][:],
                                                     start=(t == 0), stop=(t == TT - 1))
                                ps[nm] = p
                            sb = {}
                            for nm in ("qr", "qi", "kr", "ki"):
                                s = fqp.tile([128, CPC], F32, tag="s" + nm, name="s" + nm)
                                nc.vector.tensor_copy(s[:], ps[nm][:])
                                sb[nm] = s
                            t1 = fqp.tile([128, CPC], F32, tag="t1")
                            t2 = fqp.tile([128, CPC], F32, tag="t2")
                            nc.vector.tensor_mul(t1[:], sb["qr"][:], sb["kr"][:])
                            nc.vector.tensor_mul(t2[:], sb["qi"][:], sb["ki"][:])
                            nc.vector.tensor_add(pr_t[f][:], t1[:], t2[:])
                            nc.vector.tensor_mul(t1[:], sb["qi"][:], sb["kr"][:])
                            nc.vector.tensor_mul(t2[:], sb["qr"][:], sb["ki"][:])
                            nc.vector.tensor_sub(pi_t[f][:], t1[:], t2[:])
                            if f == 0:
                                # f=0 bins for the (normally zero) bias correction
                                nc.sync.dma_start(out=fq08[:], in_=sb["qr"][0:1, :])
                                nc.sync.dma_start(out=fk08[:], in_=sb["kr"][0:1, :])
                                # row 0: DC product and the packed Nyquist product
                                nc.vector.tensor_mul(pr_t[0][0:1, :], sb["qr"][0:1, :], sb["kr"][0:1, :])
                                nc.vector.tensor_mul(pi_t[0][0:1, :], sb["qi"][0:1, :], sb["ki"][0:1, :])

                # ---- stage 4: inverse DFT, max over lags 1..L-1 ----------
                # (v-path inputs prefetch underneath the inverse matmuls)
                with tc.tile_pool(name="wv6", bufs=1) as wv6, \
                     tc.tile_pool(name="vt6", bufs=1) as vt6:
                    wv_t = [wv6.tile([128, CPC], F32R, tag=f"wv{d}", name=f"wv{d}") for d in range(DT)]
                    for d in range(DT):
                        nc.sync.dma_start(out=wv_t[d][:], in_=dram["wv"][d * 128:(d + 1) * 128, :])
                    vtc = [vt6.tile([128, DT * 512], F32R, tag=f"vtc{n}", name=f"vtc{n}") for n in range(NT)]
                    for n in range(NT):
                        nc.gpsimd.dma_start(
                            out=vtc[n][:],
                            in_=dram["vT"][:, n * 512:(n + 1) * 512].rearrange("(a p) t -> p a t", p=128))

                    with tc.tile_pool(name="ci4", bufs=1) as cip, \
                         tc.tile_pool(name="ps4", bufs=8, space="PSUM") as ps4:
                        for n in range(NT):
                            if n == 0:
                                cic_t = cic0_t
                            else:
                                cic_t = [cip.tile([128, 512], F32R, tag=f"cic{f}_{n % 2}", name=f"cic{f}_{n}") for f in range(FT)]
                                for f in range(FT):
                                    eng = nc.sync if f % 2 else nc.scalar
                                    eng.dma_start(out=cic_t[f][:],
                                                  in_=dram["cic"][f * 128:(f + 1) * 128, n * 512:(n + 1) * 512])
                            cis_t = [cip.tile([128, 512], F32R, tag=f"cis{f}_{n % 2}", name=f"cis{f}_{n}") for f in range(FT)]
                            for f in range(FT):
                                eng = nc.scalar if f % 2 else nc.sync
                                eng.dma_start(out=cis_t[f][:],
                                              in_=dram["cis"][f * 128:(f + 1) * 128, n * 512:(n + 1) * 512])
                            pch = [ps4.tile([128, 512], F32, tag="inv", name=f"inv{n}_{c}") for c in range(CT)]
                            for c in range(CT):
                                for f in range(FT):
                                    nc.tensor.matmul(pch[c][:], pr_t[f][:, c * 128:(c + 1) * 128], cic_t[f][:],
                                                     start=(f == 0), stop=False)
                            for c in range(CT):
                                for f in range(FT):
                                    nc.tensor.matmul(pch[c][:], pi_t[f][:, c * 128:(c + 1) * 128], cis_t[f][:],
                                                     start=False, stop=(f == FT - 1))
                                lo = 1 if n == 0 else 0
                                nc.vector.reduce_max(mx4[c][:, n:n + 1], pch[c][:, lo:512], axis=AX)

                    for c in range(CT):
                        nc.vector.reduce_max(m_t[c][:], mx4[c][:], axis=AX)

                    # ---- stage 5: shift decision (per-head softmax) ------
                    m8 = smp.tile([8, DK], F32, tag="m8")
                    for c in range(CT):
                        nc.sync.dma_start(out=m8[2 * c:2 * c + 2, :], in_=m_t[c][:, 0:1])
                    bq8t = smp.tile([8, DK], F32, tag="bq8")
                    bk8t = smp.tile([8, DK], F32, tag="bk8")
                    nc.sync.dma_start(out=bq8t[:], in_=dram["bq8"][:, :])
                    nc.sync.dma_start(out=bk8t[:], in_=dram["bk8"][:, :])
                    # m += bk*FQ0 + bq*FK0 + L*bq*bk  (exact q/k bias effect)
                    c1 = smp.tile([8, DK], F32, tag="c1")
                    m8b = smp.tile([8, DK], F32, tag="m8b")
                    nc.vector.tensor_mul(c1[:], bk8t[:], fq08[:])
                    nc.vector.tensor_add(m8b[:], m8[:], c1[:])
                    nc.vector.tensor_mul(c1[:], bq8t[:], fk08[:])
                    nc.vector.tensor_add(m8[:], m8b[:], c1[:])
                    nc.vector.tensor_mul(c1[:], bq8t[:], bk8t[:])
                    nc.vector.scalar_tensor_tensor(m8b[:], c1[:], float(L), m8[:],
                                                   op0=OP.mult, op1=OP.add)
                    mx8 = smp.tile([8, 1], F32, tag="mx8")
                    nmx8 = smp.tile([8, 1], F32, tag="nmx8")
                    e8 = smp.tile([8, DK], F32, tag="e8")
                    s8 = smp.tile([8, 1], F32, tag="s8")
                    em8 = smp.tile([8, 1], F32, tag="em8")
                    so8 = smp.tile([8, 1], F32, tag="so8")
                    fh8 = smp.tile([8, 1], F32, tag="fh8")
                    eq8 = smp.tile([8, DK], F32, tag="eq8")
                    flag8 = smp.tile([8, DK], F32, tag="flag8")
                    nc.vector.reduce_max(mx8[:], m8b[:], axis=AX)
                    nc.vector.tensor_scalar_mul(nmx8[:], mx8[:], -1.0)
                    nc.scalar.activation(e8[:], m8b[:], mybir.ActivationFunctionType.Exp,
                                         bias=nmx8[:], scale=1.0)
                    nc.vector.reduce_sum(s8[:], e8[:], axis=AX)
                    nc.vector.reduce_max(em8[:], e8[:], axis=AX)
                    nc.vector.tensor_sub(so8[:], s8[:], em8[:])
                    # softmax weight rounds to 1.0f iff residual sum <= 2^-24
                    nc.vector.tensor_scalar(fh8[:], so8[:], float(2.0 ** -24), None, op0=OP.is_le)
                    nc.vector.tensor_scalar(eq8[:], m8b[:], mx8[:], None, op0=OP.is_equal)
                    nc.vector.tensor_scalar_mul(flag8[:], eq8[:], fh8[:])
                    nc.sync.dma_start(out=m8_d, in_=m8b[:])
                    nc.sync.dma_start(out=fl_d, in_=flag8[:])
                    fl_t = [smp.tile([128, 1], F32, tag=f"fl{c}", name=f"fl{c}") for c in range(CT)]
                    for c in range(CT):
                        nc.sync.dma_start(out=fl_t[c][:, 0:1], in_=flag8[2 * c:2 * c + 2, :])

                    # ---- stage 6+7: vpT, then in-place conditional roll --
                    with tc.tile_pool(name="vpp", bufs=1) as vpp, \
                         tc.tile_pool(name="wo8", bufs=1) as wop, \
                         tc.tile_pool(name="blend", bufs=2) as blp, \
                         tc.tile_pool(name="ps6", bufs=4, space="PSUM") as ps6:
                        vpT = [vpp.tile([128, L], F32R, tag=f"vpT{c}", name=f"vpT{c}") for c in range(CT)]
                        wo_t = [wop.tile([128, D], F32R, tag=f"wo{c}", name=f"wo{c}") for c in range(CT)]
                        for c in range(CT):
                            nc.sync.dma_start(out=wo_t[c][:], in_=dram["wo"][c * 128:(c + 1) * 128, :])
                        for c in range(CT):
                            for n in range(NT):
                                p = ps6.tile([128, 512], F32, tag="vps")
                                for d in range(DT):
                                    nc.tensor.matmul(p[:], wv_t[d][:, c * 128:(c + 1) * 128],
                                                     vtc[n][:, d * 512:(d + 1) * 512],
                                                     start=(d == 0), stop=(d == DT - 1))
                                nc.vector.tensor_copy(vpT[c][:, n * 512:(n + 1) * 512], p[:])
                            # roll-by-1 blend, in place, as soon as this c is done
                            dif = blp.tile([128, L], F32, tag="dif")
                            nc.vector.tensor_sub(dif[:, 1:L], vpT[c][:, 0:L - 1], vpT[c][:, 1:L])
                            nc.vector.tensor_sub(dif[:, 0:1], vpT[c][:, L - 1:L], vpT[c][:, 0:1])
                            nc.vector.scalar_tensor_tensor(vpT[c][:], dif[:], fl_t[c][:, 0:1], vpT[c][:],
                                                           op0=OP.mult, op1=OP.add)

                        # ---- stage 8: output projection (partial sums) ---
                        with tc.tile_pool(name="o8", bufs=3) as op8, \
                             tc.tile_pool(name="ps8", bufs=4, space="PSUM") as ps8:
                            for t in range(TT):
                                ob = op8.tile([128, D], F32, tag="ob")
                                for hh in range(2):
                                    p = ps8.tile([128, 512], F32, tag="ops")
                                    for c in range(CT):
                                        nc.tensor.matmul(p[:], vpT[c][:, t * 128:(t + 1) * 128],
                                                         wo_t[c][:, hh * 512:(hh + 1) * 512],
                                                         start=(c == 0), stop=(c == CT - 1))
                                    nc.vector.tensor_copy(ob[:, hh * 512:(hh + 1) * 512], p[:])
                                nc.sync.dma_start(out=out_d[t * 128:(t + 1) * 128, :], in_=ob[:])

    nc.compile()
    return nc


_NC_CACHE = None
_TRIG_CACHE = None


def _get_nc():
    global _NC_CACHE
    if _NC_CACHE is None:
        _NC_CACHE = build()
    return _NC_CACHE


def _trig():
    global _TRIG_CACHE
    if _TRIG_CACHE is None:
        t = np.arange(L, dtype=np.float64)
        f = np.arange(FPAD, dtype=np.float64)
        alt = np.where(t % 2 == 0, 1.0, -1.0)
        ang = np.outer(t, f) * (2.0 * np.pi / L)
        cf = np.cos(ang)
        cfs = -np.sin(ang)
        cfs[:, 0] = alt               # Nyquist row packed into the sin(0) slot
        w = np.ones(FPAD)
        w[1:] = 2.0
        angi = np.outer(f, t) * (2.0 * np.pi / L)
        cic = (w[:, None] / L) * np.cos(angi)
        cis = -(w[:, None] / L) * np.sin(angi)
        cis[0, :] = alt / L           # Nyquist contribution (weight 1, (-1)^tau)
        _TRIG_CACHE = tuple(np.ascontiguousarray(a, dtype=np.float32)
                            for a in (cf, cfs, cic, cis))
    return _TRIG_CACHE


def _run(inputs, trace=False):
    q, k, v = (np.asarray(inputs[n], np.float32) for n in ("q", "k", "v"))
    Wq, Wk, Wv, Wo = (np.asarray(inputs[n], np.float32) for n in ("Wq", "Wk", "Wv", "Wo"))
    bq, bk, bv, bo = (np.asarray(inputs[n], np.float32) for n in ("bq", "bk", "bv", "bo"))
    nc = _get_nc()
    cf, cfs, cic, cis = _trig()
    in_maps = []
    for core in range(NCORES):
        b, j = core // 2, core % 2
        cs = slice(j * CPC, (j + 1) * CPC)
        in_maps.append({
            "qT": np.ascontiguousarray(q[b].T),
            "kT": np.ascontiguousarray(k[b].T),
            "vT": np.ascontiguousarray(v[b].T),
            "wq": np.ascontiguousarray(Wq[:, cs]),
            "wk": np.ascontiguousarray(Wk[:, cs]),
            "wv": np.ascontiguousarray(Wv[:, cs]),
            "wo": np.ascontiguousarray(Wo[cs, :]),
            "cf": cf, "cfs": cfs, "cic": cic, "cis": cis,
            "bq8": np.ascontiguousarray(bq[cs].reshape(8, DK)),
            "bk8": np.ascontiguousarray(bk[cs].reshape(8, DK)),
        })
    res = run_bass_kernel_spmd(nc, in_maps, list(range(NCORES)), trace=trace)
    extra = (bv @ Wo + bo).astype(np.float32)
    out = np.empty((B, L, D), np.float32)
    for b in range(B):
        out[b] = res.results[2 * b]["out"] + res.results[2 * b + 1]["out"] + extra
    return out, res


def kernel(**inputs):
    out, _ = _run(inputs)
    return out


# revision 9
# speedup vs baseline: 1.0775x; 1.0775x over previous
"""AutoCorrelation (Autoformer-style) kernel for 8 TRN2 NeuronCores.

Reference math (B=4, L=2048, D=1024, H=16, DK=64):
  qp = q @ Wq + bq ; kp = k @ Wk + bk ; vp = v @ Wv + bv          # [B, L, D]
  corr = irfft(rfft(qp, t) * conj(rfft(kp, t)))                   # circular corr
  m[c] = max_{tau in 1..L-1} corr[tau, c]                         # top-1 value
  per head: w = softmax_{dk}(m); shift_c = int(w_c) in {0, 1}     # 1 iff w_c
                                                                  # rounds to 1.0f
  out = roll(vp, shift, axis=t) @ Wo + bo

Only the top-1 correlation value feeds the output (reference takes
delays[:, 0]), so top-k reduces to a max. Shifts are 0 except when a
softmax weight rounds to exactly 1.0f (sum of the other exp terms
< 2^-24), which happens for a handful of heads.

Sharding: core = (b, j): batch b = core//2, channel half j = core%2
(8 heads = 512 channels). Projections use column/row slices of the
weights; FFT/max/roll are head-local; the output projection produces
partial sums over the 512-channel slice which the host reduces.

DFTs are dense matmuls against precomputed trig matrices (rfft bins
1025 padded to 1152 = 9*128 partitions). All matmuls run in float32r
(~14-bit mantissa at full PE rate); the shift-decision margin for this
input is ~4 sigma above the resulting noise and is verified by test.py
against the jax reference decisions.

The host passes q/k/v pre-transposed ([D, L] per batch) so the
contraction dim of every matmul lands on SBUF partitions without
on-device transposes.
"""

import math

import numpy as np

import concourse.bacc as bacc
import concourse.mybir as mybir
from concourse import tile
from concourse.bass_utils import run_bass_kernel_spmd

B, L, D, H, DK = 4, 2048, 1024, 16, 64
NCORES = 8
CPC = D // 2        # channels per core (8 heads)
NBINS = L // 2 + 1  # 1025 rfft bins
FPAD = 1024         # Nyquist bin packed into the f=0 sin slot
FT = FPAD // 128    # 9
TT = L // 128       # 16
NT = L // 512       # 4
CT = CPC // 128     # 4
DT = D // 128       # 8
F32 = mybir.dt.float32
F32R = mybir.dt.float32r
AX = mybir.AxisListType.X
OP = mybir.AluOpType


def build():
    nc = bacc.Bacc("TRN2", target_bir_lowering=False, debug=False, num_devices=NCORES)

    dram = {}
    for name, shape, dt in [
        ("qT", [D, L], F32R), ("kT", [D, L], F32R), ("vT", [D, L], F32R),
        ("wq", [D, CPC], F32R), ("wk", [D, CPC], F32R), ("wv", [D, CPC], F32R),
        ("wo", [CPC, D], F32R),
        ("cf", [L, FPAD], F32R), ("cfs", [L, FPAD], F32R),
        ("cic", [FPAD, L], F32R), ("cis", [FPAD, L], F32R),
        ("bq8", [8, DK], F32), ("bk8", [8, DK], F32),
    ]:
        dram[name] = nc.dram_tensor(name, shape, dt, kind="ExternalInput").ap()
    out_d = nc.dram_tensor("out", [L, D], F32, kind="ExternalOutput").ap()
    m8_d = nc.dram_tensor("m8dbg", [8, DK], F32, kind="ExternalOutput").ap()
    fl_d = nc.dram_tensor("flag8dbg", [8, DK], F32, kind="ExternalOutput").ap()

    with tile.TileContext(nc) as tc:
        with tc.tile_pool(name="small", bufs=1) as smp:
            fq08 = smp.tile([8, DK], F32, tag="fq08")
            fk08 = smp.tile([8, DK], F32, tag="fk08")
            mx4 = [smp.tile([128, NT], F32, tag=f"mx4_{c}", name=f"mx4_{c}") for c in range(CT)]
            m_t = [smp.tile([128, 1], F32, tag=f"m{c}", name=f"m{c}") for c in range(CT)]

            with tc.tile_pool(name="pp", bufs=1) as ppool, \
                 tc.tile_pool(name="cip0", bufs=1) as cip0:
                pr_t = [ppool.tile([128, CPC], F32R, tag=f"pr{f}", name=f"pr{f}") for f in range(FT)]
                pi_t = [ppool.tile([128, CPC], F32R, tag=f"pi{f}", name=f"pi{f}") for f in range(FT)]

                with tc.tile_pool(name="qkp", bufs=1) as qkp, \
                     tc.tile_pool(name="cf2", bufs=2) as cfp:
                    cic0_t = [cip0.tile([128, 512], F32R, tag=f"cic0_{f}", name=f"cic0_{f}") for f in range(FT)]
                    for f in range(FT):
                        nc.scalar.dma_start(out=cic0_t[f][:],
                                            in_=dram["cic"][f * 128:(f + 1) * 128, 0:512])
                    qp_t = [qkp.tile([128, CPC], F32R, tag=f"qp{t}", name=f"qp{t}") for t in range(TT)]
                    kp_t = [qkp.tile([128, CPC], F32R, tag=f"kp{t}", name=f"kp{t}") for t in range(TT)]

                    # prefetch the first forward-trig columns during stage 1
                    cf_tiles = {}
                    def load_cf(f):
                        cfc = cfp.tile([128, L], F32R, tag="cfc", name=f"cfc{f}")
                        cfs_ = cfp.tile([128, L], F32R, tag="cfs", name=f"cfs{f}")
                        nc.sync.dma_start(
                            out=cfc[:],
                            in_=dram["cf"][:, f * 128:(f + 1) * 128].rearrange("(n p) f -> p n f", p=128))
                        nc.scalar.dma_start(
                            out=cfs_[:],
                            in_=dram["cfs"][:, f * 128:(f + 1) * 128].rearrange("(n p) f -> p n f", p=128))
                        cf_tiles[f] = (cfc, cfs_)
                    load_cf(0)

                    # ---- stage 1: projections qp/kp in [t, c] layout -----
                    with tc.tile_pool(name="w1", bufs=1) as w1, \
                         tc.tile_pool(name="xcol", bufs=2) as xcol, \
                         tc.tile_pool(name="ps1", bufs=2, space="PSUM") as ps1:
                        wq_t = [w1.tile([128, CPC], F32R, tag=f"wq{d}", name=f"wq{d}") for d in range(DT)]
                        wk_t = [w1.tile([128, CPC], F32R, tag=f"wk{d}", name=f"wk{d}") for d in range(DT)]
                        for d in range(DT):
                            nc.scalar.dma_start(out=wq_t[d][:], in_=dram["wq"][d * 128:(d + 1) * 128, :])
                            nc.scalar.dma_start(out=wk_t[d][:], in_=dram["wk"][d * 128:(d + 1) * 128, :])
                        for t in range(TT):
                            qc = xcol.tile([128, D], F32R, tag="qcol")
                            kc = xcol.tile([128, D], F32R, tag="kcol")
                            nc.sync.dma_start(
                                out=qc[:],
                                in_=dram["qT"][:, t * 128:(t + 1) * 128].rearrange("(n p) t -> p n t", p=128))
                            nc.scalar.dma_start(
                                out=kc[:],
                                in_=dram["kT"][:, t * 128:(t + 1) * 128].rearrange("(n p) t -> p n t", p=128))
                            pq = ps1.tile([128, CPC], F32, tag="pq")
                            pk = ps1.tile([128, CPC], F32, tag="pk")
                            for d in range(DT):
                                nc.tensor.matmul(pq[:], qc[:, d * 128:(d + 1) * 128], wq_t[d][:],
                                                 start=(d == 0), stop=(d == DT - 1))
                            for d in range(DT):
                                nc.tensor.matmul(pk[:], kc[:, d * 128:(d + 1) * 128], wk_t[d][:],
                                                 start=(d == 0), stop=(d == DT - 1))
                            nc.vector.tensor_copy(qp_t[t][:], pq[:])
                            nc.vector.tensor_copy(kp_t[t][:], pk[:])

                    # ---- stage 2+3: forward DFT and cross-spectrum P -----
                    # cfs column 0 carries (-1)^t (Nyquist row packed into the
                    # zero sin(0) slot); P row 0 is fixed up below.
                    with tc.tile_pool(name="fq2", bufs=2) as fqp, \
                         tc.tile_pool(name="ps2", bufs=2, space="PSUM") as ps2:
                        for f in range(FT):
                            if f + 1 < FT:
                                load_cf(f + 1)
                            cfc, cfs_ = cf_tiles.pop(f)
                            ps = {}
                            for nm, src, trig in (("qr", qp_t, cfc), ("qi", qp_t, cfs_),
                                                  ("kr", kp_t, cfc), ("ki", kp_t, cfs_)):
                                p = ps2.tile([128, CPC], F32, tag=nm, name="ps2" + nm)
                                for t in range(TT):
                                    nc.tensor.matmul(p[:], trig[:, t * 128:(t + 1) * 128], src[t][:],
                                                     start=(t == 0), stop=(t == TT - 1))
                                ps[nm] = p
                            sb = {}
                            for nm in ("qr", "qi", "kr", "ki"):
                                s = fqp.tile([128, CPC], F32, tag="s" + nm, name="s" + nm)
                                nc.vector.tensor_copy(s[:], ps[nm][:])
                                sb[nm] = s
                            t1 = fqp.tile([128, CPC], F32, tag="t1")
                            t2 = fqp.tile([128, CPC], F32, tag="t2")
                            nc.vector.tensor_mul(t1[:], sb["qr"][:], sb["kr"][:])
                            nc.vector.tensor_mul(t2[:], sb["qi"][:], sb["ki"][:])
                            nc.vector.tensor_add(pr_t[f][:], t1[:], t2[:])
                            nc.vector.tensor_mul(t1[:], sb["qi"][:], sb["kr"][:])
                            nc.vector.tensor_mul(t2[:], sb["qr"][:], sb["ki"][:])
                            nc.vector.tensor_sub(pi_t[f][:], t1[:], t2[:])
                            if f == 0:
                                # f=0 bins for the (normally zero) bias correction
                                nc.sync.dma_start(out=fq08[:], in_=sb["qr"][0:1, :])
                                nc.sync.dma_start(out=fk08[:], in_=sb["kr"][0:1, :])
                                # row 0: DC product and the packed Nyquist product
                                nc.vector.tensor_mul(pr_t[0][0:1, :], sb["qr"][0:1, :], sb["kr"][0:1, :])
                                nc.vector.tensor_mul(pi_t[0][0:1, :], sb["qi"][0:1, :], sb["ki"][0:1, :])

                # ---- stage 4: inverse DFT, max over lags 1..L-1 ----------
                # (v-path inputs prefetch underneath the inverse matmuls)
                with tc.tile_pool(name="wv6", bufs=1) as wv6, \
                     tc.tile_pool(name="vt6", bufs=1) as vt6:
                    wv_t = [wv6.tile([128, CPC], F32R, tag=f"wv{d}", name=f"wv{d}") for d in range(DT)]
                    for d in range(DT):
                        nc.sync.dma_start(out=wv_t[d][:], in_=dram["wv"][d * 128:(d + 1) * 128, :])
                    vtc = [vt6.tile([128, DT * 512], F32R, tag=f"vtc{n}", name=f"vtc{n}") for n in range(NT)]
                    for n in range(NT):
                        eng = nc.sync if n % 2 else nc.scalar
                        eng.dma_start(
                            out=vtc[n][:],
                            in_=dram["vT"][:, n * 512:(n + 1) * 512].rearrange("(a p) t -> p a t", p=128))

                    with tc.tile_pool(name="ci4", bufs=1) as cip, \
                         tc.tile_pool(name="ps4", bufs=8, space="PSUM") as ps4:
                        for n in range(NT):
                            if n == 0:
                                cic_t = cic0_t
                            else:
                                cic_t = [cip.tile([128, 512], F32R, tag=f"cic{f}_{n % 2}", name=f"cic{f}_{n}") for f in range(FT)]
                                for f in range(FT):
                                    eng = nc.sync if f % 2 else nc.scalar
                                    eng.dma_start(out=cic_t[f][:],
                                                  in_=dram["cic"][f * 128:(f + 1) * 128, n * 512:(n + 1) * 512])
                            cis_t = [cip.tile([128, 512], F32R, tag=f"cis{f}_{n % 2}", name=f"cis{f}_{n}") for f in range(FT)]
                            for f in range(FT):
                                eng = nc.scalar if f % 2 else nc.sync
                                eng.dma_start(out=cis_t[f][:],
                                              in_=dram["cis"][f * 128:(f + 1) * 128, n * 512:(n + 1) * 512])
                            pch = [ps4.tile([128, 512], F32, tag="inv", name=f"inv{n}_{c}") for c in range(CT)]
                            for c in range(CT):
                                for f in range(FT):
                                    nc.tensor.matmul(pch[c][:], pr_t[f][:, c * 128:(c + 1) * 128], cic_t[f][:],
                                                     start=(f == 0), stop=False)
                            for c in range(CT):
                                for f in range(FT):
                                    nc.tensor.matmul(pch[c][:], pi_t[f][:, c * 128:(c + 1) * 128], cis_t[f][:],
                                                     start=False, stop=(f == FT - 1))
                                lo = 1 if n == 0 else 0
                                nc.vector.reduce_max(mx4[c][:, n:n + 1], pch[c][:, lo:512], axis=AX)

                    for c in range(CT):
                        nc.vector.reduce_max(m_t[c][:], mx4[c][:], axis=AX)

                    # ---- stage 5: shift decision (per-head softmax) ------
                    m8 = smp.tile([8, DK], F32, tag="m8")
                    for c in range(CT):
                        nc.sync.dma_start(out=m8[2 * c:2 * c + 2, :], in_=m_t[c][:, 0:1])
                    bq8t = smp.tile([8, DK], F32, tag="bq8")
                    bk8t = smp.tile([8, DK], F32, tag="bk8")
                    nc.sync.dma_start(out=bq8t[:], in_=dram["bq8"][:, :])
                    nc.sync.dma_start(out=bk8t[:], in_=dram["bk8"][:, :])
                    # m += bk*FQ0 + bq*FK0 + L*bq*bk  (exact q/k bias effect)
                    c1 = smp.tile([8, DK], F32, tag="c1")
                    m8b = smp.tile([8, DK], F32, tag="m8b")
                    nc.vector.tensor_mul(c1[:], bk8t[:], fq08[:])
                    nc.vector.tensor_add(m8b[:], m8[:], c1[:])
                    nc.vector.tensor_mul(c1[:], bq8t[:], fk08[:])
                    nc.vector.tensor_add(m8[:], m8b[:], c1[:])
                    nc.vector.tensor_mul(c1[:], bq8t[:], bk8t[:])
                    nc.vector.scalar_tensor_tensor(m8b[:], c1[:], float(L), m8[:],
                                                   op0=OP.mult, op1=OP.add)
                    mx8 = smp.tile([8, 1], F32, tag="mx8")
                    nmx8 = smp.tile([8, 1], F32, tag="nmx8")
                    e8 = smp.tile([8, DK], F32, tag="e8")
                    s8 = smp.tile([8, 1], F32, tag="s8")
                    em8 = smp.tile([8, 1], F32, tag="em8")
                    so8 = smp.tile([8, 1], F32, tag="so8")
                    fh8 = smp.tile([8, 1], F32, tag="fh8")
                    eq8 = smp.tile([8, DK], F32, tag="eq8")
                    flag8 = smp.tile([8, DK], F32, tag="flag8")
                    nc.vector.reduce_max(mx8[:], m8b[:], axis=AX)
                    nc.vector.tensor_scalar_mul(nmx8[:], mx8[:], -1.0)
                    nc.scalar.activation(e8[:], m8b[:], mybir.ActivationFunctionType.Exp,
                                         bias=nmx8[:], scale=1.0)
                    nc.vector.reduce_sum(s8[:], e8[:], axis=AX)
                    nc.vector.reduce_max(em8[:], e8[:], axis=AX)
                    nc.vector.tensor_sub(so8[:], s8[:], em8[:])
                    # softmax weight rounds to 1.0f iff residual sum <= 2^-24
                    nc.vector.tensor_scalar(fh8[:], so8[:], float(2.0 ** -24), None, op0=OP.is_le)
                    nc.vector.tensor_scalar(eq8[:], m8b[:], mx8[:], None, op0=OP.is_equal)
                    nc.vector.tensor_scalar_mul(flag8[:], eq8[:], fh8[:])
                    nc.sync.dma_start(out=m8_d, in_=m8b[:])
                    nc.sync.dma_start(out=fl_d, in_=flag8[:])
                    fl_t = [smp.tile([128, 1], F32, tag=f"fl{c}", name=f"fl{c}") for c in range(CT)]
                    for c in range(CT):
                        nc.sync.dma_start(out=fl_t[c][:, 0:1], in_=flag8[2 * c:2 * c + 2, :])

                    # ---- stage 6+7: vpT, then in-place conditional roll --
                    with tc.tile_pool(name="vpp", bufs=1) as vpp, \
                         tc.tile_pool(name="wo8", bufs=1) as wop, \
                         tc.tile_pool(name="blend", bufs=2) as blp, \
                         tc.tile_pool(name="ps6", bufs=4, space="PSUM") as ps6:
                        vpT = [vpp.tile([128, L], F32R, tag=f"vpT{c}", name=f"vpT{c}") for c in range(CT)]
                        wo_t = [wop.tile([128, D], F32R, tag=f"wo{c}", name=f"wo{c}") for c in range(CT)]
                        for c in range(CT):
                            nc.sync.dma_start(out=wo_t[c][:], in_=dram["wo"][c * 128:(c + 1) * 128, :])
                        for c in range(CT):
                            for n in range(NT):
                                p = ps6.tile([128, 512], F32, tag="vps")
                                for d in range(DT):
                                    nc.tensor.matmul(p[:], wv_t[d][:, c * 128:(c + 1) * 128],
                                                     vtc[n][:, d * 512:(d + 1) * 512],
                                                     start=(d == 0), stop=(d == DT - 1))
                                nc.vector.tensor_copy(vpT[c][:, n * 512:(n + 1) * 512], p[:])
                            # roll-by-1 blend, in place, as soon as this c is done
                            dif = blp.tile([128, L], F32, tag="dif")
                            nc.vector.tensor_sub(dif[:, 1:L], vpT[c][:, 0:L - 1], vpT[c][:, 1:L])
                            nc.vector.tensor_sub(dif[:, 0:1], vpT[c][:, L - 1:L], vpT[c][:, 0:1])
                            nc.vector.scalar_tensor_tensor(vpT[c][:], dif[:], fl_t[c][:, 0:1], vpT[c][:],
                                                           op0=OP.mult, op1=OP.add)

                        # ---- stage 8: output projection (partial sums) ---
                        with tc.tile_pool(name="o8", bufs=3) as op8, \
                             tc.tile_pool(name="ps8", bufs=4, space="PSUM") as ps8:
                            for t in range(TT):
                                ob = op8.tile([128, D], F32, tag="ob")
                                for hh in range(2):
                                    p = ps8.tile([128, 512], F32, tag="ops")
                                    for c in range(CT):
                                        nc.tensor.matmul(p[:], vpT[c][:, t * 128:(t + 1) * 128],
                                                         wo_t[c][:, hh * 512:(hh + 1) * 512],
                                                         start=(c == 0), stop=(c == CT - 1))
                                    nc.vector.tensor_copy(ob[:, hh * 512:(hh + 1) * 512], p[:])
                                nc.sync.dma_start(out=out_d[t * 128:(t + 1) * 128, :], in_=ob[:])

    nc.compile()
    return nc


_NC_CACHE = None
_TRIG_CACHE = None


def _get_nc():
    global _NC_CACHE
    if _NC_CACHE is None:
        _NC_CACHE = build()
    return _NC_CACHE


def _trig():
    global _TRIG_CACHE
    if _TRIG_CACHE is None:
        t = np.arange(L, dtype=np.float64)
        f = np.arange(FPAD, dtype=np.float64)
        alt = np.where(t % 2 == 0, 1.0, -1.0)
        ang = np.outer(t, f) * (2.0 * np.pi / L)
        cf = np.cos(ang)
        cfs = -np.sin(ang)
        cfs[:, 0] = alt               # Nyquist row packed into the sin(0) slot
        w = np.ones(FPAD)
        w[1:] = 2.0
        angi = np.outer(f, t) * (2.0 * np.pi / L)
        cic = (w[:, None] / L) * np.cos(angi)
        cis = -(w[:, None] / L) * np.sin(angi)
        cis[0, :] = alt / L           # Nyquist contribution (weight 1, (-1)^tau)
        _TRIG_CACHE = tuple(np.ascontiguousarray(a, dtype=np.float32)
                            for a in (cf, cfs, cic, cis))
    return _TRIG_CACHE


def _run(inputs, trace=False):
    q, k, v = (np.asarray(inputs[n], np.float32) for n in ("q", "k", "v"))
    Wq, Wk, Wv, Wo = (np.asarray(inputs[n], np.float32) for n in ("Wq", "Wk", "Wv", "Wo"))
    bq, bk, bv, bo = (np.asarray(inputs[n], np.float32) for n in ("bq", "bk", "bv", "bo"))
    nc = _get_nc()
    cf, cfs, cic, cis = _trig()
    in_maps = []
    for core in range(NCORES):
        b, j = core // 2, core % 2
        cs = slice(j * CPC, (j + 1) * CPC)
        in_maps.append({
            "qT": np.ascontiguousarray(q[b].T),
            "kT": np.ascontiguousarray(k[b].T),
            "vT": np.ascontiguousarray(v[b].T),
            "wq": np.ascontiguousarray(Wq[:, cs]),
            "wk": np.ascontiguousarray(Wk[:, cs]),
            "wv": np.ascontiguousarray(Wv[:, cs]),
            "wo": np.ascontiguousarray(Wo[cs, :]),
            "cf": cf, "cfs": cfs, "cic": cic, "cis": cis,
            "bq8": np.ascontiguousarray(bq[cs].reshape(8, DK)),
            "bk8": np.ascontiguousarray(bk[cs].reshape(8, DK)),
        })
    res = run_bass_kernel_spmd(nc, in_maps, list(range(NCORES)), trace=trace)
    extra = (bv @ Wo + bo).astype(np.float32)
    out = np.empty((B, L, D), np.float32)
    for b in range(B):
        out[b] = res.results[2 * b]["out"] + res.results[2 * b + 1]["out"] + extra
    return out, res


def kernel(**inputs):
    out, _ = _run(inputs)
    return out


# revision 10
# speedup vs baseline: 1.1059x; 1.0264x over previous
"""AutoCorrelation (Autoformer-style) kernel for 8 TRN2 NeuronCores.

Reference math (B=4, L=2048, D=1024, H=16, DK=64):
  qp = q @ Wq + bq ; kp = k @ Wk + bk ; vp = v @ Wv + bv          # [B, L, D]
  corr = irfft(rfft(qp, t) * conj(rfft(kp, t)))                   # circular corr
  m[c] = max_{tau in 1..L-1} corr[tau, c]                         # top-1 value
  per head: w = softmax_{dk}(m); shift_c = int(w_c) in {0, 1}     # 1 iff w_c
                                                                  # rounds to 1.0f
  out = roll(vp, shift, axis=t) @ Wo + bo

Only the top-1 correlation value feeds the output (reference takes
delays[:, 0]), so top-k reduces to a max. Shifts are 0 except when a
softmax weight rounds to exactly 1.0f (sum of the other exp terms
< 2^-24), which happens for a handful of heads.

Sharding: core = (b, j): batch b = core//2, channel half j = core%2
(8 heads = 512 channels). Projections use column/row slices of the
weights; FFT/max/roll are head-local; the output projection produces
partial sums over the 512-channel slice which the host reduces.

DFTs are dense matmuls against precomputed trig matrices (rfft bins
1025 padded to 1152 = 9*128 partitions). All matmuls run in float32r
(~14-bit mantissa at full PE rate); the shift-decision margin for this
input is ~4 sigma above the resulting noise and is verified by test.py
against the jax reference decisions.

The host passes q/k/v pre-transposed ([D, L] per batch) so the
contraction dim of every matmul lands on SBUF partitions without
on-device transposes.
"""

import math

import numpy as np

import concourse.bacc as bacc
import concourse.mybir as mybir
from concourse import tile
from concourse.bass_utils import run_bass_kernel_spmd

B, L, D, H, DK = 4, 2048, 1024, 16, 64
NCORES = 8
CPC = D // 2        # channels per core (8 heads)
NBINS = L // 2 + 1  # 1025 rfft bins
FPAD = 1024         # Nyquist bin packed into the f=0 sin slot
FT = FPAD // 128    # 9
TT = L // 128       # 16
NT = L // 512       # 4
CT = CPC // 128     # 4
DT = D // 128       # 8
F32 = mybir.dt.float32
F32R = mybir.dt.float32r
AX = mybir.AxisListType.X
OP = mybir.AluOpType


def build():
    nc = bacc.Bacc("TRN2", target_bir_lowering=False, debug=False, num_devices=NCORES)

    dram = {}
    for name, shape, dt in [
        ("qT", [D, L], F32R), ("kT", [D, L], F32R), ("vT", [D, L], F32R),
        ("wq", [D, CPC], F32R), ("wk", [D, CPC], F32R), ("wv", [D, CPC], F32R),
        ("wo", [CPC, D], F32R),
        ("cf", [L, FPAD], F32R), ("cfs", [L, FPAD], F32R),
        ("cic", [FPAD, L], F32R), ("cis", [FPAD, L], F32R),
        ("bq8", [8, DK], F32), ("bk8", [8, DK], F32),
    ]:
        dram[name] = nc.dram_tensor(name, shape, dt, kind="ExternalInput").ap()
    out_d = nc.dram_tensor("out", [L, D], F32, kind="ExternalOutput").ap()
    m8_d = nc.dram_tensor("m8dbg", [8, DK], F32, kind="ExternalOutput").ap()
    fl_d = nc.dram_tensor("flag8dbg", [8, DK], F32, kind="ExternalOutput").ap()

    with tile.TileContext(nc) as tc:
        with tc.tile_pool(name="small", bufs=1) as smp:
            fq08 = smp.tile([8, DK], F32, tag="fq08")
            fk08 = smp.tile([8, DK], F32, tag="fk08")
            mx4 = [smp.tile([128, NT], F32, tag=f"mx4_{c}", name=f"mx4_{c}") for c in range(CT)]
            m_t = [smp.tile([128, 1], F32, tag=f"m{c}", name=f"m{c}") for c in range(CT)]

            with tc.tile_pool(name="pp", bufs=1) as ppool, \
                 tc.tile_pool(name="cip0", bufs=1) as cip0:
                pr_t = [ppool.tile([128, CPC], F32R, tag=f"pr{f}", name=f"pr{f}") for f in range(FT)]
                pi_t = [ppool.tile([128, CPC], F32R, tag=f"pi{f}", name=f"pi{f}") for f in range(FT)]

                with tc.tile_pool(name="qkp", bufs=1) as qkp, \
                     tc.tile_pool(name="cf2", bufs=2) as cfp:
                    cic0_t = [cip0.tile([128, 512], F32R, tag=f"cic0_{f}", name=f"cic0_{f}") for f in range(FT)]
                    for f in range(FT):
                        nc.scalar.dma_start(out=cic0_t[f][:],
                                            in_=dram["cic"][f * 128:(f + 1) * 128, 0:512])
                    qp_t = [qkp.tile([128, CPC], F32R, tag=f"qp{t}", name=f"qp{t}") for t in range(TT)]
                    kp_t = [qkp.tile([128, CPC], F32R, tag=f"kp{t}", name=f"kp{t}") for t in range(TT)]

                    # prefetch the first forward-trig columns during stage 1
                    cf_tiles = {}
                    def load_cf(f):
                        cfc = cfp.tile([128, L], F32R, tag="cfc", name=f"cfc{f}")
                        cfs_ = cfp.tile([128, L], F32R, tag="cfs", name=f"cfs{f}")
                        nc.sync.dma_start(
                            out=cfc[:],
                            in_=dram["cf"][:, f * 128:(f + 1) * 128].rearrange("(n p) f -> p n f", p=128))
                        nc.scalar.dma_start(
                            out=cfs_[:],
                            in_=dram["cfs"][:, f * 128:(f + 1) * 128].rearrange("(n p) f -> p n f", p=128))
                        cf_tiles[f] = (cfc, cfs_)
                    load_cf(0)

                    # ---- stage 1: projections qp/kp in [t, c] layout -----
                    with tc.tile_pool(name="w1", bufs=1) as w1, \
                         tc.tile_pool(name="xcol", bufs=2) as xcol, \
                         tc.tile_pool(name="ps1", bufs=2, space="PSUM") as ps1:
                        wq_t = [w1.tile([128, CPC], F32R, tag=f"wq{d}", name=f"wq{d}") for d in range(DT)]
                        wk_t = [w1.tile([128, CPC], F32R, tag=f"wk{d}", name=f"wk{d}") for d in range(DT)]
                        for d in range(DT):
                            nc.scalar.dma_start(out=wq_t[d][:], in_=dram["wq"][d * 128:(d + 1) * 128, :])
                            nc.scalar.dma_start(out=wk_t[d][:], in_=dram["wk"][d * 128:(d + 1) * 128, :])
                        for t in range(TT):
                            qc = xcol.tile([128, D], F32R, tag="qcol")
                            kc = xcol.tile([128, D], F32R, tag="kcol")
                            nc.sync.dma_start(
                                out=qc[:],
                                in_=dram["qT"][:, t * 128:(t + 1) * 128].rearrange("(n p) t -> p n t", p=128))
                            nc.scalar.dma_start(
                                out=kc[:],
                                in_=dram["kT"][:, t * 128:(t + 1) * 128].rearrange("(n p) t -> p n t", p=128))
                            pq = ps1.tile([128, CPC], F32, tag="pq")
                            pk = ps1.tile([128, CPC], F32, tag="pk")
                            for d in range(DT):
                                nc.tensor.matmul(pq[:], qc[:, d * 128:(d + 1) * 128], wq_t[d][:],
                                                 start=(d == 0), stop=(d == DT - 1))
                            for d in range(DT):
                                nc.tensor.matmul(pk[:], kc[:, d * 128:(d + 1) * 128], wk_t[d][:],
                                                 start=(d == 0), stop=(d == DT - 1))
                            nc.vector.tensor_copy(qp_t[t][:], pq[:])
                            nc.vector.tensor_copy(kp_t[t][:], pk[:])

                    # ---- stage 2+3: forward DFT and cross-spectrum P -----
                    # cfs column 0 carries (-1)^t (Nyquist row packed into the
                    # zero sin(0) slot); P row 0 is fixed up below.
                    with tc.tile_pool(name="fq2", bufs=2) as fqp, \
                         tc.tile_pool(name="ps2", bufs=2, space="PSUM") as ps2:
                        for f in range(FT):
                            if f + 1 < FT:
                                load_cf(f + 1)
                            cfc, cfs_ = cf_tiles.pop(f)
                            ps = {}
                            for nm, src, trig in (("qr", qp_t, cfc), ("qi", qp_t, cfs_),
                                                  ("kr", kp_t, cfc), ("ki", kp_t, cfs_)):
                                p = ps2.tile([128, CPC], F32, tag=nm, name="ps2" + nm)
                                for t in range(TT):
                                    nc.tensor.matmul(p[:], trig[:, t * 128:(t + 1) * 128], src[t][:],
                                                     start=(t == 0), stop=(t == TT - 1))
                                ps[nm] = p
                            sb = {}
                            for nm in ("qr", "qi", "kr", "ki"):
                                s = fqp.tile([128, CPC], F32, tag="s" + nm, name="s" + nm)
                                nc.vector.tensor_copy(s[:], ps[nm][:])
                                sb[nm] = s
                            t1 = fqp.tile([128, CPC], F32, tag="t1")
                            t2 = fqp.tile([128, CPC], F32, tag="t2")
                            nc.vector.tensor_mul(t1[:], sb["qr"][:], sb["kr"][:])
                            nc.vector.tensor_mul(t2[:], sb["qi"][:], sb["ki"][:])
                            nc.vector.tensor_add(pr_t[f][:], t1[:], t2[:])
                            nc.vector.tensor_mul(t1[:], sb["qi"][:], sb["kr"][:])
                            nc.vector.tensor_mul(t2[:], sb["qr"][:], sb["ki"][:])
                            nc.vector.tensor_sub(pi_t[f][:], t1[:], t2[:])
                            if f == 0:
                                # f=0 bins for the (normally zero) bias correction
                                nc.sync.dma_start(out=fq08[:], in_=sb["qr"][0:1, :])
                                nc.sync.dma_start(out=fk08[:], in_=sb["kr"][0:1, :])
                                # row 0: DC product and the packed Nyquist product
                                nc.vector.tensor_mul(pr_t[0][0:1, :], sb["qr"][0:1, :], sb["kr"][0:1, :])
                                nc.vector.tensor_mul(pi_t[0][0:1, :], sb["qi"][0:1, :], sb["ki"][0:1, :])

                # ---- stage 4: inverse DFT, max over lags 1..L-1 ----------
                # (v-path inputs prefetch underneath the inverse matmuls)
                with tc.tile_pool(name="wv6", bufs=1) as wv6, \
                     tc.tile_pool(name="vt6", bufs=1) as vt6:
                    wv_t = [wv6.tile([128, CPC], F32R, tag=f"wv{d}", name=f"wv{d}") for d in range(DT)]
                    vtc = [vt6.tile([128, DT * 512], F32R, tag=f"vtc{n}", name=f"vtc{n}") for n in range(NT)]

                    with tc.tile_pool(name="ci4", bufs=1) as cip, \
                         tc.tile_pool(name="ps4", bufs=8, space="PSUM") as ps4:
                        # inverse trig loads, most-urgent first: chunk 0/1
                        # before the v-path prefetch, then chunk n+1 inside
                        # the loop (FIFO order on the DMA queues is priority)
                        cic_all = [cic0_t] + [None] * (NT - 1)
                        cis_all = [None] * NT

                        def load_ci(n):
                            if n >= NT:
                                return
                            if cic_all[n] is None:
                                cic_all[n] = [cip.tile([128, 512], F32R, tag=f"cic{f}_{n % 2}", name=f"cic{f}_{n}") for f in range(FT)]
                                for f in range(FT):
                                    eng = nc.sync if f % 2 else nc.scalar
                                    eng.dma_start(out=cic_all[n][f][:],
                                                  in_=dram["cic"][f * 128:(f + 1) * 128, n * 512:(n + 1) * 512])
                            cis_all[n] = [cip.tile([128, 512], F32R, tag=f"cis{f}_{n % 2}", name=f"cis{f}_{n}") for f in range(FT)]
                            for f in range(FT):
                                eng = nc.scalar if f % 2 else nc.sync
                                eng.dma_start(out=cis_all[n][f][:],
                                              in_=dram["cis"][f * 128:(f + 1) * 128, n * 512:(n + 1) * 512])
                        load_ci(0)
                        load_ci(1)
                        for d in range(DT):
                            eng = nc.sync if d % 2 else nc.scalar
                            eng.dma_start(out=wv_t[d][:], in_=dram["wv"][d * 128:(d + 1) * 128, :])
                        for n in range(NT):
                            eng = nc.sync if n % 2 else nc.scalar
                            eng.dma_start(
                                out=vtc[n][:],
                                in_=dram["vT"][:, n * 512:(n + 1) * 512].rearrange("(a p) t -> p a t", p=128))
                        for n in range(NT):
                            load_ci(n + 2)
                            cic_t = cic_all[n]
                            cis_t = cis_all[n]
                            pch = [ps4.tile([128, 512], F32, tag="inv", name=f"inv{n}_{c}") for c in range(CT)]
                            for c in range(CT):
                                for f in range(FT):
                                    nc.tensor.matmul(pch[c][:], pr_t[f][:, c * 128:(c + 1) * 128], cic_t[f][:],
                                                     start=(f == 0), stop=False)
                            for c in range(CT):
                                for f in range(FT):
                                    nc.tensor.matmul(pch[c][:], pi_t[f][:, c * 128:(c + 1) * 128], cis_t[f][:],
                                                     start=False, stop=(f == FT - 1))
                                lo = 1 if n == 0 else 0
                                nc.vector.reduce_max(mx4[c][:, n:n + 1], pch[c][:, lo:512], axis=AX)

                    for c in range(CT):
                        nc.vector.reduce_max(m_t[c][:], mx4[c][:], axis=AX)

                    # ---- stage 5: shift decision (per-head softmax) ------
                    m8 = smp.tile([8, DK], F32, tag="m8")
                    for c in range(CT):
                        nc.sync.dma_start(out=m8[2 * c:2 * c + 2, :], in_=m_t[c][:, 0:1])
                    bq8t = smp.tile([8, DK], F32, tag="bq8")
                    bk8t = smp.tile([8, DK], F32, tag="bk8")
                    nc.sync.dma_start(out=bq8t[:], in_=dram["bq8"][:, :])
                    nc.sync.dma_start(out=bk8t[:], in_=dram["bk8"][:, :])
                    # m += bk*FQ0 + bq*FK0 + L*bq*bk  (exact q/k bias effect)
                    c1 = smp.tile([8, DK], F32, tag="c1")
                    m8b = smp.tile([8, DK], F32, tag="m8b")
                    nc.vector.tensor_mul(c1[:], bk8t[:], fq08[:])
                    nc.vector.tensor_add(m8b[:], m8[:], c1[:])
                    nc.vector.tensor_mul(c1[:], bq8t[:], fk08[:])
                    nc.vector.tensor_add(m8[:], m8b[:], c1[:])
                    nc.vector.tensor_mul(c1[:], bq8t[:], bk8t[:])
                    nc.vector.scalar_tensor_tensor(m8b[:], c1[:], float(L), m8[:],
                                                   op0=OP.mult, op1=OP.add)
                    mx8 = smp.tile([8, 1], F32, tag="mx8")
                    nmx8 = smp.tile([8, 1], F32, tag="nmx8")
                    e8 = smp.tile([8, DK], F32, tag="e8")
                    s8 = smp.tile([8, 1], F32, tag="s8")
                    em8 = smp.tile([8, 1], F32, tag="em8")
                    so8 = smp.tile([8, 1], F32, tag="so8")
                    fh8 = smp.tile([8, 1], F32, tag="fh8")
                    eq8 = smp.tile([8, DK], F32, tag="eq8")
                    flag8 = smp.tile([8, DK], F32, tag="flag8")
                    nc.vector.reduce_max(mx8[:], m8b[:], axis=AX)
                    nc.vector.tensor_scalar_mul(nmx8[:], mx8[:], -1.0)
                    nc.scalar.activation(e8[:], m8b[:], mybir.ActivationFunctionType.Exp,
                                         bias=nmx8[:], scale=1.0)
                    nc.vector.reduce_sum(s8[:], e8[:], axis=AX)
                    nc.vector.reduce_max(em8[:], e8[:], axis=AX)
                    nc.vector.tensor_sub(so8[:], s8[:], em8[:])
                    # softmax weight rounds to 1.0f iff residual sum <= 2^-24
                    nc.vector.tensor_scalar(fh8[:], so8[:], float(2.0 ** -24), None, op0=OP.is_le)
                    nc.vector.tensor_scalar(eq8[:], m8b[:], mx8[:], None, op0=OP.is_equal)
                    nc.vector.tensor_scalar_mul(flag8[:], eq8[:], fh8[:])
                    nc.sync.dma_start(out=m8_d, in_=m8b[:])
                    nc.sync.dma_start(out=fl_d, in_=flag8[:])
                    fl_t = [smp.tile([128, 1], F32, tag=f"fl{c}", name=f"fl{c}") for c in range(CT)]
                    for c in range(CT):
                        nc.sync.dma_start(out=fl_t[c][:, 0:1], in_=flag8[2 * c:2 * c + 2, :])

                    # ---- stage 6+7: vpT, then in-place conditional roll --
                    with tc.tile_pool(name="vpp", bufs=1) as vpp, \
                         tc.tile_pool(name="wo8", bufs=1) as wop, \
                         tc.tile_pool(name="blend", bufs=2) as blp, \
                         tc.tile_pool(name="ps6", bufs=4, space="PSUM") as ps6:
                        vpT = [vpp.tile([128, L], F32R, tag=f"vpT{c}", name=f"vpT{c}") for c in range(CT)]
                        wo_t = [wop.tile([128, D], F32R, tag=f"wo{c}", name=f"wo{c}") for c in range(CT)]
                        for c in range(CT):
                            nc.sync.dma_start(out=wo_t[c][:], in_=dram["wo"][c * 128:(c + 1) * 128, :])
                        for c in range(CT):
                            for n in range(NT):
                                p = ps6.tile([128, 512], F32, tag="vps")
                                for d in range(DT):
                                    nc.tensor.matmul(p[:], wv_t[d][:, c * 128:(c + 1) * 128],
                                                     vtc[n][:, d * 512:(d + 1) * 512],
                                                     start=(d == 0), stop=(d == DT - 1))
                                nc.vector.tensor_copy(vpT[c][:, n * 512:(n + 1) * 512], p[:])
                            # roll-by-1 blend, in place, as soon as this c is done
                            dif = blp.tile([128, L], F32, tag="dif")
                            nc.vector.tensor_sub(dif[:, 1:L], vpT[c][:, 0:L - 1], vpT[c][:, 1:L])
                            nc.vector.tensor_sub(dif[:, 0:1], vpT[c][:, L - 1:L], vpT[c][:, 0:1])
                            nc.vector.scalar_tensor_tensor(vpT[c][:], dif[:], fl_t[c][:, 0:1], vpT[c][:],
                                                           op0=OP.mult, op1=OP.add)

                        # ---- stage 8: output projection (partial sums) ---
                        with tc.tile_pool(name="o8", bufs=3) as op8, \
                             tc.tile_pool(name="ps8", bufs=4, space="PSUM") as ps8:
                            for t in range(TT):
                                ob = op8.tile([128, D], F32, tag="ob")
                                for hh in range(2):
                                    p = ps8.tile([128, 512], F32, tag="ops")
                                    for c in range(CT):
                                        nc.tensor.matmul(p[:], vpT[c][:, t * 128:(t + 1) * 128],
                                                         wo_t[c][:, hh * 512:(hh + 1) * 512],
                                                         start=(c == 0), stop=(c == CT - 1))
                                    nc.vector.tensor_copy(ob[:, hh * 512:(hh + 1) * 512], p[:])
                                nc.sync.dma_start(out=out_d[t * 128:(t + 1) * 128, :], in_=ob[:])

    nc.compile()
    return nc


_NC_CACHE = None
_TRIG_CACHE = None


def _get_nc():
    global _NC_CACHE
    if _NC_CACHE is None:
        _NC_CACHE = build()
    return _NC_CACHE


def _trig():
    global _TRIG_CACHE
    if _TRIG_CACHE is None:
        t = np.arange(L, dtype=np.float64)
        f = np.arange(FPAD, dtype=np.float64)
        alt = np.where(t % 2 == 0, 1.0, -1.0)
        ang = np.outer(t, f) * (2.0 * np.pi / L)
        cf = np.cos(ang)
        cfs = -np.sin(ang)
        cfs[:, 0] = alt               # Nyquist row packed into the sin(0) slot
        w = np.ones(FPAD)
        w[1:] = 2.0
        angi = np.outer(f, t) * (2.0 * np.pi / L)
        cic = (w[:, None] / L) * np.cos(angi)
        cis = -(w[:, None] / L) * np.sin(angi)
        cis[0, :] = alt / L           # Nyquist contribution (weight 1, (-1)^tau)
        _TRIG_CACHE = tuple(np.ascontiguousarray(a, dtype=np.float32)
                            for a in (cf, cfs, cic, cis))
    return _TRIG_CACHE


def _run(inputs, trace=False):
    q, k, v = (np.asarray(inputs[n], np.float32) for n in ("q", "k", "v"))
    Wq, Wk, Wv, Wo = (np.asarray(inputs[n], np.float32) for n in ("Wq", "Wk", "Wv", "Wo"))
    bq, bk, bv, bo = (np.asarray(inputs[n], np.float32) for n in ("bq", "bk", "bv", "bo"))
    nc = _get_nc()
    cf, cfs, cic, cis = _trig()
    in_maps = []
    for core in range(NCORES):
        b, j = core // 2, core % 2
        cs = slice(j * CPC, (j + 1) * CPC)
        in_maps.append({
            "qT": np.ascontiguousarray(q[b].T),
            "kT": np.ascontiguousarray(k[b].T),
            "vT": np.ascontiguousarray(v[b].T),
            "wq": np.ascontiguousarray(Wq[:, cs]),
            "wk": np.ascontiguousarray(Wk[:, cs]),
            "wv": np.ascontiguousarray(Wv[:, cs]),
            "wo": np.ascontiguousarray(Wo[cs, :]),
            "cf": cf, "cfs": cfs, "cic": cic, "cis": cis,
            "bq8": np.ascontiguousarray(bq[cs].reshape(8, DK)),
            "bk8": np.ascontiguousarray(bk[cs].reshape(8, DK)),
        })
    res = run_bass_kernel_spmd(nc, in_maps, list(range(NCORES)), trace=trace)
    extra = (bv @ Wo + bo).astype(np.float32)
    out = np.empty((B, L, D), np.float32)
    for b in range(B):
        out[b] = res.results[2 * b]["out"] + res.results[2 * b + 1]["out"] + extra
    return out, res


def kernel(**inputs):
    out, _ = _run(inputs)
    return out
